# revision 13
# baseline (speedup 1.0000x reference)
"""Sparse 3x3x3 conv (C_in=C_out=1) over N=2M voxels in a 256^3 grid.

Strategy (dense_cnn): densify the sparse voxels into a zero-padded 258^3
grid laid out [z, x, y]; run the whole 27-tap stencil on the tensor
engine.  The z-axis taps live in banded 128x128 weight matrices (contract
dim = z), the 9 (dx,dy) taps become free-dim AP offsets on the moving
operand, PSUM-accumulated.  The two z-chunks use full 128-row contract
windows; the 2 missing boundary rows per chunk are covered by one extra
"patch" matmul whose 18 rhs partitions hold the boundary rows pre-shifted
by each of the 9 (dx,dy) offsets.  x is sharded across the 8 cores (32
output x-slices each); outputs are gathered back to point order on host.
"""

import os as _os

import numpy as np

import concourse.bass as bass
import concourse.mybir as mybir
import concourse.tile as tile
from concourse import bacc
from concourse.bass_utils import run_bass_kernel_spmd

G = 256              # grid extent
P = G + 2            # padded extent
NCORES = 8
XS = G // NCORES     # 32 output x-slices per core
XW = XS + 2          # 34-slice input window (x halo)
ROWF = XW * P        # 8772 elements per z-row of a core's slab
BANKS = XS * G // 512          # 16 PSUM tiles of 512 (= 2 x-rows) per chunk
OFF9 = [(dx, dy) for dx in (-1, 0, 1) for dy in (-1, 0, 1)]

_PE_DT_NAME = _os.environ.get("BASS_PE_DT", "float16")
PE_DT = getattr(mybir.dt, _PE_DT_NAME)
NP_DT = np.float32             # scatter dtype on host


def _np_store(dt):
    if dt == mybir.dt.float16:
        return np.float16
    if dt == mybir.dt.bfloat16:
        import ml_dtypes
        return ml_dtypes.bfloat16
    return np.float32


STORE_DT = _np_store(PE_DT)


def _build_nc(iters=1, fd=512, do_in=True, do_out=True):
    n_banks = XS * G // fd
    xpb = fd // G                  # x-rows per bank
    nc = bacc.Bacc("TRN2", target_bir_lowering=False, debug=False)
    slab = nc.dram_tensor("slab", [P, XW, P], PE_DT, kind="ExternalInput")
    wsb = nc.dram_tensor("wsb", [128, 9, 128], PE_DT, kind="ExternalInput")
    wpat = nc.dram_tensor("wpat", [18, 128], PE_DT, kind="ExternalInput")
    out = nc.dram_tensor("out", [G, XS * G], mybir.dt.float32, kind="ExternalOutput")
    slab_flat = slab.ap().rearrange("z x y -> (z x y)")

    with tile.TileContext(nc) as tc:
        with (
            tc.tile_pool(name="w", bufs=1) as wp,
            tc.tile_pool(name="inp", bufs=2) as ip,
            tc.tile_pool(name="pat", bufs=2) as tp,
            tc.tile_pool(name="ps", bufs=8 * 512 // fd, space="PSUM") as pp,
            tc.tile_pool(name="ob", bufs=6) as op,
        ):
            wt = wp.tile([128, 9, 128], PE_DT)
            nc.sync.dma_start(out=wt[:], in_=wsb[:])
            wq = wp.tile([18, 128], PE_DT)
            nc.sync.dma_start(out=wq[:], in_=wpat[:])

            def load_tiles(z0):
                it = ip.tile([128, XW, P], PE_DT, tag="inp", name="it")
                nc.sync.dma_start(out=it[:], in_=slab[z0:z0 + 128])
                # patch rows z0+128, z0+129 pre-shifted by the 9 offsets:
                # partition (r*9 + dxi*3 + dyi) holds row z0+128+r shifted
                # by dx*P + (dyi-1); only lanes [P+1, ROWF-P-1) are read
                # by the patch matmuls, so only those are filled.
                pt = tp.tile([18, XW, P], PE_DT, tag="pat", name="pt")
                ptf = pt.rearrange("p x y -> p (x y)")
                L = ROWF - 2 * P - 2
                for r in (0, 1):
                    for dxi, dx in enumerate((-1, 0, 1)):
                        src = bass.AP(
                            tensor=slab_flat.tensor,
                            offset=(z0 + 128 + r) * ROWF + dx * P + P,
                            ap=[[1, 3], [1, L]],
                        )
                        nc.sync.dma_start(
                            out=ptf[r * 9 + dxi * 3:r * 9 + dxi * 3 + 3,
                                    P + 1:P + 1 + L],
                            in_=src,
                        )
                return it, pt

            hoisted = None
            if not do_in:
                hoisted = {z0: load_tiles(z0) for z0 in (0, 128)}

            def body(_i=None):
                for z0 in (0, 128):
                    it, pt = hoisted[z0] if hoisted else load_tiles(z0)
                    for b in range(n_banks):
                        ps = pp.tile([128, fd], mybir.dt.float32, tag="ps",
                                     name="ps")
                        xb = xpb * b
                        for j, (dx, dy) in enumerate(OFF9):
                            nc.tensor.matmul(
                                ps[:],
                                wt[:, j, :],
                                it[:, xb + 1 + dx:xb + 1 + xpb + dx,
                                   1 + dy:G + 1 + dy],
                                start=(j == 0),
                                stop=False,
                            )
                        nc.tensor.matmul(
                            ps[:],
                            wq[:],
                            pt[:, xb + 1:xb + 1 + xpb, 1:G + 1],
                            start=False,
                            stop=True,
                        )
                        if do_out or b == n_banks - 1:
                            sb = op.tile([128, fd], mybir.dt.float32, tag="ob",
                                         name="sb")
                            if b % 2 == 0:
                                nc.scalar.copy(out=sb[:], in_=ps[:])
                            else:
                                nc.vector.tensor_copy(sb[:], ps[:])
                            nc.scalar.dma_start(
                                out=out[z0:z0 + 128, b * fd:(b + 1) * fd],
                                in_=sb[:],
                            )

            if iters == 1:
                body()
            else:
                with tc.For_i(0, iters, 1):
                    body()
    nc.finalize()
    return nc


_NC_CACHE = {}


def _get_nc(iters=1, **kw):
    key = (iters, tuple(sorted(kw.items())))
    if key not in _NC_CACHE:
        _NC_CACHE[key] = _build_nc(iters, **kw)
    return _NC_CACHE[key]


def _make_wsb(W):
    W27 = np.asarray(W, dtype=NP_DT).reshape(27)
    wsb = np.zeros((128, 9, 128), dtype=NP_DT)
    for j in range(9):
        for k in range(3):
            c = np.arange(0, 128 - k)
            wsb[c + k, j, c] = W27[j * 3 + k]
    wpat = np.zeros((18, 128), dtype=NP_DT)
    for j in range(9):
        wpat[j, 126] = W27[j * 3 + 2]
        wpat[j, 127] = W27[j * 3 + 1]
        wpat[9 + j, 127] = W27[j * 3 + 2]
    return wsb.astype(STORE_DT), wpat.astype(STORE_DT)


def _make_in_maps(coords, feats, W):
    x = coords[:, 0].astype(np.int64)
    y = coords[:, 1].astype(np.int64)
    z = coords[:, 2].astype(np.int64)
    Dp = np.zeros((P, P, P), dtype=NP_DT)          # [z_pad, x_pad, y_pad]
    # reversed order: on (unexpected) duplicate coords the first occurrence
    # wins, matching the reference's stable argsort + searchsorted lookup
    Dp[z[::-1] + 1, x[::-1] + 1, y[::-1] + 1] = \
        np.asarray(feats, dtype=NP_DT)[::-1, 0]
    if STORE_DT is not np.float32:
        Dp = Dp.astype(STORE_DT)
    wsb, wpat = _make_wsb(W)
    in_maps = [
        {"slab": np.ascontiguousarray(Dp[:, XS * c:XS * c + XW, :]),
         "wsb": wsb, "wpat": wpat}
        for c in range(NCORES)
    ]
    return in_maps, x, y, z


def kernel(coords, feats, W):
    coords = np.asarray(coords)
    in_maps, x, y, z = _make_in_maps(coords, feats, W)
    nc = _get_nc(1)
    res = run_bass_kernel_spmd(nc, in_maps, list(range(NCORES)))
    Ofull = np.concatenate(
        [res.results[c]["out"].reshape(G, XS, G) for c in range(NCORES)], axis=1
    )                                               # [z, x, y]
    return Ofull[z, x, y].astype(np.float32).reshape(-1, 1)
